# revision 12
# baseline (speedup 1.0000x reference)
"""AttentionBlock (GroupNorm + QKV + 8-head spatial attention + proj + residual)
on 8 Trainium2 NeuronCores.

Sharding: 16 head-batches (B=2 x NH=8) split 2-per-core; cores 0-3 take batch
0, cores 4-7 batch 1.  Per core:
  - x arrives as fp8e4m3 [512, 4096]; GroupNorm statistics computed on-chip
    (bn_stats per channel, group-combine via tiny mask matmuls on the PE),
  - GroupNorm affine folded into the QKV weights (W' = W*A per channel,
    bias' = W@B + qkv_b); QKV matmuls run in fp8 DoubleRow mode (contracting
    2x128 channels per pass),
  - scores in f32r [s,t] layout (K=128 with the other head's k rows zeroed),
  - exp is split between the ACT engine (Exp -> fp8 out) and the DVE
    (Schraudolph: bits = rint(A*sc+B) saturating to uint8, bit-identical
    cost, ~3% sawtooth err) writing E as fp8e4m3 pairs [128, 2, 1024],
  - a_plus = vT @ E in fp8 DoubleRow mode (contracts 2 s-chunks of 128 per
    pass; v transposed out of QKV with an extra ones-column so softmax
    denominators are free; av lags exp by one pair so the PE never stalls),
  - per-t normalization via partition-scattered reciprocal (DMA reshapes the
    denominator row to [8,128] so DVE reciprocal uses partition parallelism),
  - partial projection proj_w[:, head_cols] @ a emitted per t-stripe, lagged.
Host sums the 4 partials per batch, adds proj_b + proj_w@vbias' term... (vb
is kept on-device in vT, so host only adds proj_b) and the residual.
"""

import math
import numpy as np

import concourse.bacc as bacc
import concourse.tile as tile
from concourse import mybir
from concourse.bass_utils import run_bass_kernel_spmd

B, C = 2, 512
L = 64 * 64           # 4096
NH = 8                # heads total
CH = 64               # channels per head
G = 32                # groups
EPS = 1e-5
N_CORES = 8
HEADS_PER_CORE = 2

F32 = mybir.dt.float32
F32R = mybir.dt.float32r
F8 = mybir.dt.float8e4
U8 = mybir.dt.uint8
AF = mybir.ActivationFunctionType
ALU = mybir.AluOpType
PM = mybir.MatmulPerfMode

TSUP = 1024           # t-stripe width
NT = L // TSUP        # 4 stripes
SJ = 32               # number of 128-wide s-chunks

# exp-domain shift (softmax-invariant; keeps fp8 E in range)
EBIAS = 2.5
# DVE schraudolph constants: bits = rint(sc*SCH_A + SCH_B), sc = raw q.k
SCH_A = 8.0 * (1.0 / math.log(2.0)) * 0.125
SCH_B = 56.0 - 8.0 * EBIAS * (1.0 / math.log(2.0)) - 2.8

# which j-chunks the DVE computes (rest on ACT); ~37.5%
DVE_J = frozenset((1, 4, 7, 10, 12, 15))


def _f(ap):
    return ap.bitcast(F32)


_PROGRAM = None


def build_program():
    nc = bacc.Bacc()
    x8b = nc.declare_dram_parameter("x8b", [C, L], F8, isOutput=False).ap()
    gmask = nc.declare_dram_parameter("gmask", [128, 4, G], F32R, isOutput=False).ap()
    bmask = nc.declare_dram_parameter("bmask", [G, 4, 128], F32R, isOutput=False).ap()
    gamma4 = nc.declare_dram_parameter("gamma4", [4, 128], F32, isOutput=False).ap()
    beta4 = nc.declare_dram_parameter("beta4", [4, 128], F32, isOutput=False).ap()
    wqT = nc.declare_dram_parameter("wqT", [C, 128], F32R, isOutput=False).ap()
    wkT = nc.declare_dram_parameter("wkT", [C, 128], F32R, isOutput=False).ap()
    wvT = nc.declare_dram_parameter("wvT", [C, 130], F32R, isOutput=False).ap()
    qb = nc.declare_dram_parameter("qb", [128], F32, isOutput=False).ap()
    kb = nc.declare_dram_parameter("kb", [128], F32, isOutput=False).ap()
    vb = nc.declare_dram_parameter("vb", [130], F32, isOutput=False).ap()
    pwT = nc.declare_dram_parameter("pwT", [128, C], F32R, isOutput=False).ap()
    part = nc.declare_dram_parameter("part", [C, L], F32, isOutput=True).ap()

    with tile.TileContext(nc) as tc:
        with (
            tc.tile_pool(name="consts", bufs=1) as consts,
            tc.tile_pool(name="big", bufs=1) as big,
            tc.tile_pool(name="work", bufs=2) as work,
            tc.tile_pool(name="ps", bufs=1, space="PSUM") as ps,
        ):
            # ---- constants into SBUF ----
            sb_gmask = consts.tile([128, 4, G], F32R)
            nc.sync.dma_start(out=sb_gmask, in_=gmask)
            sb_bmask = consts.tile([G, 4, 128], F32R)
            nc.sync.dma_start(out=sb_bmask, in_=bmask)
            sb_gamma = consts.tile([128, 4], F32)
            nc.sync.dma_start(out=sb_gamma, in_=gamma4.rearrange("t p -> p t"))
            sb_beta = consts.tile([128, 4], F32)
            nc.sync.dma_start(out=sb_beta, in_=beta4.rearrange("t p -> p t"))
            sb_wq = consts.tile([128, 4, 128], F32R)
            nc.sync.dma_start(out=sb_wq, in_=wqT.rearrange("(kk p) m -> p kk m", p=128))
            sb_wk = consts.tile([128, 4, 128], F32R)
            nc.sync.dma_start(out=sb_wk, in_=wkT.rearrange("(kk p) m -> p kk m", p=128))
            sb_wv = consts.tile([128, 4, 130], F32R)
            nc.sync.dma_start(out=sb_wv, in_=wvT.rearrange("(kk p) m -> p kk m", p=128))
            sb_pw = consts.tile([128, C], F32R)
            nc.sync.dma_start(out=sb_pw, in_=pwT)
            sb_qb = consts.tile([128, 1], F32)
            nc.sync.dma_start(out=sb_qb, in_=qb.unsqueeze(1))
            sb_kb = consts.tile([128, 1], F32)
            nc.sync.dma_start(out=sb_kb, in_=kb.unsqueeze(1))
            sb_vb = consts.tile([1, 130], F32)
            nc.sync.dma_start(out=sb_vb, in_=vb.unsqueeze(0))
            eps32 = consts.tile([32, 1], F32)
            nc.vector.memset(eps32, EPS)
            ebias_t = consts.tile([128, 1], F32)
            nc.vector.memset(ebias_t, -EBIAS)
            mh0 = consts.tile([128, 1], F32)
            nc.vector.memset(mh0[0:64, :], 1.0)
            nc.vector.memset(mh0[64:128, :], 0.0)
            mh1 = consts.tile([128, 1], F32)
            nc.vector.memset(mh1[0:64, :], 0.0)
            nc.vector.memset(mh1[64:128, :], 1.0)

            # vt8: av stationary, [p, m(16), i(2), h(2), 128]; cols 0:64 v,
            # col 64 ones (denominator), 65:127 zero-pad
            vt8 = big.tile([128, 16, 2, 2, 128], F8)
            nc.gpsimd.memset(vt8, 0.0)

            # ---- load x8 ----
            x8 = big.tile([128, 4, L], F8)
            x8r = x8b.rearrange("(t p) l -> p t l", p=128)
            stats = work.tile([128, 2, 8, 6], F32, bufs=1)
            sums = work.tile([128, 2, 2, 8], F32, bufs=1)
            scr = work.tile([128, 512], F32, tag="scr", bufs=2, name="scr")
            for s in range(8):
                ns = slice(s * 512, (s + 1) * 512)
                nc.sync.dma_start(out=x8[:, :, ns], in_=x8r[:, :, ns])
                for t in range(2):
                    nc.vector.bn_stats(out=stats[:, t, s, :], in_=x8[:, t, ns])
                for t in range(2, 4):
                    scr_t = work.tile([128, 512], F32, tag="scr", bufs=2, name="scr")
                    nc.scalar.activation(out=scr_t, in_=x8[:, t, ns], func=AF.Copy,
                                         accum_out=sums[:, t - 2, 0, s:s + 1])
                    scr_t2 = work.tile([128, 512], F32, tag="scr", bufs=2, name="scr")
                    nc.scalar.activation(out=scr_t2, in_=x8[:, t, ns], func=AF.Square,
                                         accum_out=sums[:, t - 2, 1, s:s + 1])
            mv = work.tile([128, 2, 2], F32, bufs=1)
            for t in range(2):
                nc.vector.bn_aggr(out=mv[:, t, :], in_=stats[:, t, :, :])
            # per-channel [mean, E[x^2]]
            stats2 = work.tile([128, 4, 2], F32R, bufs=1)
            msq = work.tile([128, 2, 1], F32, bufs=1)
            nc.vector.tensor_copy(out=stats2[:, 0:2, 0:1], in_=mv[:, :, 0:1])
            nc.vector.tensor_mul(msq, mv[:, :, 0:1], mv[:, :, 0:1])
            nc.vector.tensor_add(stats2[:, 0:2, 1:2], mv[:, :, 1:2], msq)
            ssum = work.tile([128, 2, 2], F32, bufs=1)
            nc.vector.tensor_reduce(out=ssum.rearrange("p a b -> p (a b)").unsqueeze(2),
                                    in_=sums.rearrange("p a b s -> p (a b) s"),
                                    axis=mybir.AxisListType.X, op=ALU.add)
            nc.vector.tensor_scalar_mul(out=stats2[:, 2:4, :],
                                        in0=ssum.rearrange("p a b -> p a b"),
                                        scalar1=1.0 / 4096.0)
            # group stats via mask matmul: [32, 2] = (mean_g, E[x^2]_g)
            gps = ps.tile([32, 2], F32, tag="apl")
            for t in range(4):
                nc.tensor.matmul(
                    gps, sb_gmask[:, t, :], stats2[:, t, :],
                    start=(t == 0), stop=(t == 3),
                )
            gs = work.tile([32, 2], F32, bufs=1)
            nc.vector.tensor_copy(out=gs, in_=gps)
            msqg = work.tile([32, 1], F32, bufs=1)
            varg = work.tile([32, 1], F32, bufs=1)
            nc.vector.tensor_mul(msqg, gs[:, 0:1], gs[:, 0:1])
            nc.vector.tensor_sub(varg, gs[:, 1:2], msqg)
            # rstd = exp(-0.5*ln(var+eps))
            lng = work.tile([32, 1], F32, bufs=1)
            nc.scalar.activation(out=lng, in_=varg, func=AF.Ln, bias=eps32, scale=1.0)
            rstdg = work.tile([32, 1], F32, bufs=1)
            nc.scalar.activation(out=rstdg, in_=lng, func=AF.Exp, scale=-0.5)
            gstats2 = work.tile([32, 2], F32R, bufs=1)
            nc.vector.tensor_copy(out=gstats2[:, 0:1], in_=gs[:, 0:1])
            nc.vector.tensor_copy(out=gstats2[:, 1:2], in_=rstdg)

            # ---- per-channel affine A, Bs  (hid = x*A + Bs) ----
            A_all = work.tile([128, 4], F32, bufs=1)
            Bcol = work.tile([128, 4, 2], F32R, bufs=1)
            for t in range(4):
                cst = ps.tile([128, 2], F32, tag="sc", bufs=3)
                nc.tensor.matmul(
                    cst, sb_bmask[:, t, :], gstats2, start=True, stop=True
                )
                nc.vector.tensor_mul(A_all[:, t:t + 1], cst[:, 1:2], sb_gamma[:, t:t + 1])
                tmp = work.tile([128, 1], F32, tag="tmp")
                nc.vector.tensor_mul(tmp, cst[:, 0:1], A_all[:, t:t + 1])
                nc.vector.tensor_sub(Bcol[:, t, :], sb_beta[:, t:t + 1].broadcast_to([128, 2]), tmp.broadcast_to([128, 2]))

            # ---- fold affine into QKV weights ----
            # bias' = W^T @ Bs + b (reads original f32r W), then fp8 W' = W*A
            cq_ps = ps.tile([128, 2], F32, tag="sc", bufs=3)
            ck_ps = ps.tile([128, 2], F32, tag="apl")
            cv_ps = ps.tile([1, 130], F32, tag="apl")
            for t in range(4):
                nc.tensor.matmul(cq_ps, sb_wq[:, t, :], Bcol[:, t, :],
                                 start=(t == 0), stop=(t == 3))
                nc.tensor.matmul(ck_ps, sb_wk[:, t, :], Bcol[:, t, :],
                                 start=(t == 0), stop=(t == 3))
                nc.tensor.matmul(cv_ps, Bcol[:, t, 0:1], sb_wv[:, t, :],
                                 start=(t == 0), stop=(t == 3))
            qc = consts.tile([128, 1], F32)
            nc.vector.tensor_add(qc, cq_ps[:, 0:1], sb_qb)
            kc = consts.tile([128, 1], F32)
            nc.vector.tensor_add(kc, ck_ps[:, 0:1], sb_kb)
            kc0 = consts.tile([128, 1], F32)
            nc.vector.tensor_mul(kc0, kc, mh0)
            kc1 = consts.tile([128, 1], F32)
            nc.vector.tensor_mul(kc1, kc, mh1)
            vrow = work.tile([1, 130], F32, bufs=1)
            nc.vector.tensor_add(vrow, cv_ps[:, 0:130], sb_vb)
            vbc = consts.tile([128, 130], F32)
            nc.gpsimd.partition_broadcast(vbc, vrow)
            # fp8 folded weights: [p, d(2), i(2), m]
            wq8 = consts.tile([128, 2, 2, 128], F8)
            wk8 = consts.tile([128, 2, 2, 128], F8)
            wv8 = consts.tile([128, 2, 2, 130], F8)
            for t in range(4):
                d, i = t // 2, t % 2
                nc.vector.tensor_scalar_mul(
                    out=wq8[:, d, i, :], in0=_f(sb_wq[:, t, :]), scalar1=A_all[:, t:t + 1])
                nc.vector.tensor_scalar_mul(
                    out=wk8[:, d, i, :], in0=_f(sb_wk[:, t, :]), scalar1=A_all[:, t:t + 1])
                nc.vector.tensor_scalar_mul(
                    out=wv8[:, d, i, :], in0=_f(sb_wv[:, t, :]), scalar1=A_all[:, t:t + 1])

            # ---- QKV (fp8 DoubleRow: contraction 512 = 2 passes x 256) ----
            q2 = big.tile([128, L], F32R)
            k2z = [big.tile([128, L], F32R, name="k2z0"),
                   big.tile([128, L], F32R, name="k2z1")]
            for n in range(8):
                ns = slice(n * 512, (n + 1) * 512)
                qp = ps.tile([128, 512], F32, tag="sc", bufs=3, name="qp")
                for d in range(2):
                    nc.tensor.matmul(qp, wq8[:, d], x8[:, 2 * d:2 * d + 2, ns],
                                     start=(d == 0), stop=(d == 1),
                                     perf_mode=PM.DoubleRow)
                if n % 2 == 0:
                    nc.scalar.activation(out=q2[:, ns], in_=qp, func=AF.Identity,
                                         bias=qc, scale=1.0)
                else:
                    nc.vector.tensor_scalar_add(out=q2[:, ns], in0=qp, scalar1=qc)
                kp = ps.tile([128, 512], F32, tag="sc", bufs=3, name="kp")
                for d in range(2):
                    nc.tensor.matmul(kp, wk8[:, d], x8[:, 2 * d:2 * d + 2, ns],
                                     start=(d == 0), stop=(d == 1),
                                     perf_mode=PM.DoubleRow)
                # (k + kc) masked per head (scores contract K=128)
                if n % 2 == 0:
                    nc.scalar.activation(out=k2z[0][:, ns], in_=kp, func=AF.Identity,
                                         bias=kc0, scale=mh0)
                    nc.vector.tensor_scalar(out=k2z[1][:, ns], in0=kp, scalar1=kc,
                                            scalar2=mh1, op0=ALU.add, op1=ALU.mult)
                else:
                    nc.vector.tensor_scalar(out=k2z[0][:, ns], in0=kp, scalar1=kc,
                                            scalar2=mh0, op0=ALU.add, op1=ALU.mult)
                    nc.scalar.activation(out=k2z[1][:, ns], in_=kp, func=AF.Identity,
                                         bias=kc1, scale=mh1)

            def emit_vp(j):
                js = slice(j * 128, (j + 1) * 128)
                vp = ps.tile([128, 130], F32, tag="sc", bufs=3, name="vp")
                for d in range(2):
                    nc.tensor.matmul(vp, x8[:, 2 * d:2 * d + 2, js], wv8[:, d],
                                     start=(d == 0), stop=(d == 1),
                                     perf_mode=PM.DoubleRow)
                m, i = j // 2, j % 2
                # heads' 65-col blocks (v + ones col) into padded stationary
                nc.vector.tensor_add(
                    vt8[:, m, i, :, 0:65],
                    vp.rearrange("p (h c) -> p h c", h=2),
                    vbc.rearrange("p (h c) -> p h c", h=2))

            for j in range(SJ):
                emit_vp(j)

            # ---- attention ----
            a_cat = big.tile([128, L], F32R, tag="xt")
            dbat = work.tile([8, 128], F32, tag="dbat", bufs=2, name="dbat")
            rrow = work.tile([1, TSUP], F32, tag="rrow", bufs=2, name="rrow")

            def emit_normalize(key, acp_t):
                hh, ts_idx = key
                tb = ts_idx * TSUP
                hsn = slice(CH * hh, CH * (hh + 1))
                rt = work.tile([8, 128], F32, tag="rt", bufs=2, name="rt")
                nc.vector.reciprocal(rt, dbat)
                nc.sync.dma_start(
                    out=rrow.rearrange("o (p f) -> o p f", p=8), in_=rt)
                rbc = work.tile([64, TSUP], F32, tag="rbc", bufs=2, name="rbc")
                for g in range(2):
                    gsl = slice(g * 512, (g + 1) * 512)
                    nc.gpsimd.partition_broadcast(rbc[:, gsl], rrow[:, gsl])
                    nc.gpsimd.tensor_mul(
                        a_cat[hsn, tb + g * 512:tb + (g + 1) * 512],
                        acp_t[0:64, gsl], rbc[:, gsl])

            def emit_proj_piece(ts_idx, piece):
                # piece 0..7 -> (n, m): n-outer so the first a_cat half suffices
                tb = ts_idx * TSUP
                n, m = piece // 4, piece % 4
                ms = slice(m * 128, (m + 1) * 128)
                ns = slice(tb + n * 512, tb + (n + 1) * 512)
                pp = ps.tile([128, 512], F32, tag="sc", bufs=3, name="pp")
                nc.tensor.matmul(pp, sb_pw[:, ms], a_cat[:, ns],
                                 start=True, stop=True)
                pt = work.tile([128, 512], F32, tag="pt", bufs=2, name="pt")
                if m % 2 == 0:
                    nc.scalar.activation(out=pt, in_=pp, func=AF.Copy)
                else:
                    nc.vector.tensor_copy(out=pt, in_=pp)
                nc.sync.dma_start(out=part[ms, ns], in_=pt)

            pending_norm = None
            for tsup in range(NT):
                t0 = tsup * TSUP
                for h in range(HEADS_PER_CORE):
                    apl = ps.tile([128, TSUP], F32, tag="apl", name="apl")
                    E8s = {}
                    E8 = None
                    for j in range(SJ + 4):
                        if j == 4 and pending_norm is not None:
                            emit_normalize(*pending_norm)
                            pending_norm = None
                        if h == 1 and tsup > 0 and j in (17, 19, 21, 23, 25, 27, 29, 31):
                            emit_proj_piece(tsup - 1, (j - 17) // 2)
                        if j < SJ:
                            js = slice(j * 128, (j + 1) * 128)
                            if j % 2 == 0:
                                E8s[j // 2] = work.tile([128, 2, TSUP], F8, bufs=4, name="E8")
                            E8 = E8s[j // 2]
                            sc = ps.tile([128, TSUP], F32, tag="sc", bufs=3, name="sc")
                            nc.tensor.matmul(sc[:, 0:512], k2z[h][:, js],
                                             q2[:, t0:t0 + 512], start=True, stop=True)
                            nc.tensor.matmul(sc[:, 512:1024], k2z[h][:, js],
                                             q2[:, t0 + 512:t0 + 1024],
                                             start=True, stop=True)
                            pidx = j // 2
                            on_dve = (pidx % 2 == 0) == (j % 2 == 1)
                            if on_dve:
                                nc.vector.tensor_scalar(
                                    out=E8[:, j % 2, :].bitcast(U8), in0=sc,
                                    scalar1=SCH_A, scalar2=SCH_B,
                                    op0=ALU.mult, op1=ALU.add)
                            else:
                                nc.scalar.activation(
                                    out=E8[:, j % 2, :], in_=sc, func=AF.Exp,
                                    scale=0.125, bias=ebias_t)
                        # av lagged two pairs
                        if j % 2 == 1 and j >= 5:
                            pav = (j - 5) // 2
                            Ep = E8s.pop(pav)
                            vst = vt8[:, pav, :, h, :]
                            nc.tensor.matmul(apl[:, 0:512], vst, Ep[:, :, 0:512],
                                             start=(pav == 0), stop=(pav == 15),
                                             perf_mode=PM.DoubleRow)
                            nc.tensor.matmul(apl[:, 512:1024], vst, Ep[:, :, 512:1024],
                                             start=(pav == 0), stop=(pav == 15),
                                             perf_mode=PM.DoubleRow)
                    # move a_plus off PSUM (rows 0:64 = channels, 64 = denom)
                    acp = work.tile([65, TSUP], F32, tag="acp", bufs=3, name="acp")
                    nc.scalar.activation(out=acp, in_=apl[0:65, :], func=AF.Copy)
                    # denominator row -> [8, 128] for partition-parallel recip
                    nc.sync.dma_start(
                        out=dbat,
                        in_=acp[64:65, :].rearrange("o (p f) -> o p f", p=8))
                    pending_norm = ((h, tsup), acp)
            emit_normalize(*pending_norm)
            for piece in range(8):
                emit_proj_piece(NT - 1, piece)

    nc.compile()
    return nc


def get_program():
    global _PROGRAM
    if _PROGRAM is None:
        _PROGRAM = build_program()
    return _PROGRAM


def make_in_maps(x, norm_w, norm_b, qkv_w, qkv_b, proj_w):
    """Build the 8 per-core input maps from full inputs."""
    import ml_dtypes
    f = np.float32
    x8 = np.ascontiguousarray(x.reshape(B, C, L)).astype(ml_dtypes.float8_e4m3fn)

    gmask = np.zeros((128, 4, G), dtype=f)
    bmask = np.zeros((G, 4, 128), dtype=f)
    for t in range(4):
        for p in range(128):
            g = (t * 128 + p) // 16
            gmask[p, t, g] = 1.0 / 16.0
            bmask[g, t, p] = 1.0
    gamma4 = np.ascontiguousarray(norm_w.reshape(4, 128), dtype=f)
    beta4 = np.ascontiguousarray(norm_b.reshape(4, 128), dtype=f)

    in_maps = []
    for cid in range(N_CORES):
        b = cid // 4
        h0 = 2 * (cid % 4)
        h1 = h0 + 1
        qrows = list(range(192 * h0, 192 * h0 + 64)) + list(range(192 * h1, 192 * h1 + 64))
        krows = [r + 64 for r in qrows]
        v0 = list(range(192 * h0 + 128, 192 * h0 + 192))
        v1 = list(range(192 * h1 + 128, 192 * h1 + 192))
        wqT = np.ascontiguousarray(qkv_w[qrows, :].T, dtype=f)
        wkT = np.ascontiguousarray(qkv_w[krows, :].T, dtype=f)
        wvT = np.zeros((C, 130), dtype=f)
        wvT[:, 0:64] = qkv_w[v0, :].T
        wvT[:, 65:129] = qkv_w[v1, :].T
        qbv = np.ascontiguousarray(qkv_b[qrows], dtype=f)
        kbv = np.ascontiguousarray(qkv_b[krows], dtype=f)
        vbv = np.zeros((130,), dtype=f)
        vbv[0:64] = qkv_b[v0]
        vbv[65:129] = qkv_b[v1]
        vbv[64] = 1.0    # softmax-denominator ones columns (weight cols there are 0)
        vbv[129] = 1.0
        ch_cols = list(range(64 * h0, 64 * h0 + 64)) + list(range(64 * h1, 64 * h1 + 64))
        pwT = np.ascontiguousarray(proj_w[:, ch_cols].T, dtype=f)
        in_maps.append({
            "x8b": x8[b], "gmask": gmask, "bmask": bmask,
            "gamma4": gamma4, "beta4": beta4,
            "wqT": wqT, "wkT": wkT, "wvT": wvT,
            "qb": qbv, "kb": kbv, "vb": vbv, "pwT": pwT,
        })
    return in_maps


def kernel(x, norm_w, norm_b, qkv_w, qkv_b, proj_w, proj_b, _trace=False):
    x = np.asarray(x, dtype=np.float32)
    in_maps = make_in_maps(x, np.asarray(norm_w), np.asarray(norm_b),
                           np.asarray(qkv_w), np.asarray(qkv_b), np.asarray(proj_w))
    nc = get_program()
    res = run_bass_kernel_spmd(nc, in_maps, list(range(N_CORES)), trace=_trace)
    hout = np.zeros((B, C, L), dtype=np.float32)
    for cid in range(N_CORES):
        hout[cid // 4] += res.results[cid]["part"]
    hout += np.asarray(proj_b, dtype=np.float32)[None, :, None]
    out = x + hout.reshape(x.shape)
    if _trace:
        return out.astype(np.float32), res
    return out.astype(np.float32)


# revision 13
# speedup vs baseline: 1.0292x; 1.0292x over previous
"""AttentionBlock (GroupNorm + QKV + 8-head spatial attention + proj + residual)
on 8 Trainium2 NeuronCores.

Sharding: 16 head-batches (B=2 x NH=8) split 2-per-core; cores 0-3 take batch
0, cores 4-7 batch 1.  Per core:
  - x arrives as fp8e4m3 [512, 4096]; GroupNorm statistics computed on-chip
    (bn_stats per channel, group-combine via tiny mask matmuls on the PE),
  - GroupNorm affine folded into the QKV weights (W' = W*A per channel,
    bias' = W@B + qkv_b); QKV matmuls run in fp8 DoubleRow mode (contracting
    2x128 channels per pass),
  - scores in f32r [s,t] layout (K=128 with the other head's k rows zeroed),
  - exp is split between the ACT engine (Exp -> fp8 out) and the DVE
    (Schraudolph: bits = rint(A*sc+B) saturating to uint8, bit-identical
    cost, ~3% sawtooth err) writing E as fp8e4m3 pairs [128, 2, 1024],
  - a_plus = vT @ E in fp8 DoubleRow mode (contracts 2 s-chunks of 128 per
    pass; v transposed out of QKV with an extra ones-column so softmax
    denominators are free; av lags exp by one pair so the PE never stalls),
  - per-t normalization via partition-scattered reciprocal (DMA reshapes the
    denominator row to [8,128] so DVE reciprocal uses partition parallelism),
  - partial projection proj_w[:, head_cols] @ a emitted per t-stripe, lagged.
Host sums the 4 partials per batch, adds proj_b + proj_w@vbias' term... (vb
is kept on-device in vT, so host only adds proj_b) and the residual.
"""

import math
import numpy as np

import concourse.bacc as bacc
import concourse.tile as tile
from concourse import mybir
from concourse.bass_utils import run_bass_kernel_spmd

B, C = 2, 512
L = 64 * 64           # 4096
NH = 8                # heads total
CH = 64               # channels per head
G = 32                # groups
EPS = 1e-5
N_CORES = 8
HEADS_PER_CORE = 2

F32 = mybir.dt.float32
F32R = mybir.dt.float32r
F8 = mybir.dt.float8e4
U8 = mybir.dt.uint8
AF = mybir.ActivationFunctionType
ALU = mybir.AluOpType
PM = mybir.MatmulPerfMode

TSUP = 1024           # t-stripe width
NT = L // TSUP        # 4 stripes
SJ = 32               # number of 128-wide s-chunks

# exp-domain shift (softmax-invariant; keeps fp8 E in range)
EBIAS = 2.5
# DVE schraudolph constants: bits = rint(sc*SCH_A + SCH_B), sc = raw q.k
SCH_A = 8.0 * (1.0 / math.log(2.0)) * 0.125
SCH_B = 56.0 - 8.0 * EBIAS * (1.0 / math.log(2.0)) - 2.8

# which j-chunks the DVE computes (rest on ACT); ~37.5%
DVE_J = frozenset((1, 4, 7, 10, 12, 15))


def _f(ap):
    return ap.bitcast(F32)


_PROGRAM = None


def build_program():
    nc = bacc.Bacc()
    x8b = nc.declare_dram_parameter("x8b", [C, L], F8, isOutput=False).ap()
    gmask = nc.declare_dram_parameter("gmask", [128, 4, G], F32R, isOutput=False).ap()
    bmask = nc.declare_dram_parameter("bmask", [G, 4, 128], F32R, isOutput=False).ap()
    gamma4 = nc.declare_dram_parameter("gamma4", [4, 128], F32, isOutput=False).ap()
    beta4 = nc.declare_dram_parameter("beta4", [4, 128], F32, isOutput=False).ap()
    wqT = nc.declare_dram_parameter("wqT", [C, 128], F32R, isOutput=False).ap()
    wkT = nc.declare_dram_parameter("wkT", [C, 128], F32R, isOutput=False).ap()
    wvT = nc.declare_dram_parameter("wvT", [C, 130], F32R, isOutput=False).ap()
    qb = nc.declare_dram_parameter("qb", [128], F32, isOutput=False).ap()
    kb = nc.declare_dram_parameter("kb", [128], F32, isOutput=False).ap()
    vb = nc.declare_dram_parameter("vb", [130], F32, isOutput=False).ap()
    pwT = nc.declare_dram_parameter("pwT", [128, C], F32R, isOutput=False).ap()
    part = nc.declare_dram_parameter("part", [C, L], F32, isOutput=True).ap()

    with tile.TileContext(nc) as tc:
        with (
            tc.tile_pool(name="consts", bufs=1) as consts,
            tc.tile_pool(name="big", bufs=1) as big,
            tc.tile_pool(name="work", bufs=2) as work,
            tc.tile_pool(name="ps", bufs=1, space="PSUM") as ps,
        ):
            # ---- constants into SBUF ----
            sb_gmask = consts.tile([128, 4, G], F32R)
            nc.sync.dma_start(out=sb_gmask, in_=gmask)
            sb_bmask = consts.tile([G, 4, 128], F32R)
            nc.sync.dma_start(out=sb_bmask, in_=bmask)
            sb_gamma = consts.tile([128, 4], F32)
            nc.sync.dma_start(out=sb_gamma, in_=gamma4.rearrange("t p -> p t"))
            sb_beta = consts.tile([128, 4], F32)
            nc.sync.dma_start(out=sb_beta, in_=beta4.rearrange("t p -> p t"))
            sb_wq = consts.tile([128, 4, 128], F32R)
            nc.sync.dma_start(out=sb_wq, in_=wqT.rearrange("(kk p) m -> p kk m", p=128))
            sb_wk = consts.tile([128, 4, 128], F32R)
            nc.sync.dma_start(out=sb_wk, in_=wkT.rearrange("(kk p) m -> p kk m", p=128))
            sb_wv = consts.tile([128, 4, 130], F32R)
            nc.sync.dma_start(out=sb_wv, in_=wvT.rearrange("(kk p) m -> p kk m", p=128))
            sb_pw = consts.tile([128, C], F32R)
            nc.sync.dma_start(out=sb_pw, in_=pwT)
            sb_qb = consts.tile([128, 1], F32)
            nc.sync.dma_start(out=sb_qb, in_=qb.unsqueeze(1))
            sb_kb = consts.tile([128, 1], F32)
            nc.sync.dma_start(out=sb_kb, in_=kb.unsqueeze(1))
            sb_vb = consts.tile([1, 130], F32)
            nc.sync.dma_start(out=sb_vb, in_=vb.unsqueeze(0))
            eps32 = consts.tile([32, 1], F32)
            nc.vector.memset(eps32, EPS)
            ebias_t = consts.tile([128, 1], F32)
            nc.vector.memset(ebias_t, -EBIAS)
            mh0 = consts.tile([128, 1], F32)
            nc.vector.memset(mh0[0:64, :], 1.0)
            nc.vector.memset(mh0[64:128, :], 0.0)
            mh1 = consts.tile([128, 1], F32)
            nc.vector.memset(mh1[0:64, :], 0.0)
            nc.vector.memset(mh1[64:128, :], 1.0)

            # vt8: av stationary, [p, m(16), i(2), h(2), 128]; cols 0:64 v,
            # col 64 ones (denominator), 65:127 zero-pad
            vt8 = big.tile([128, 16, 2, 2, 128], F8)
            nc.gpsimd.memset(vt8, 0.0)

            # ---- load x8 ----
            x8 = big.tile([128, 4, L], F8)
            x8r = x8b.rearrange("(t p) l -> p t l", p=128)
            stats = work.tile([128, 3, 8, 6], F32, bufs=1)
            sums = work.tile([128, 1, 2, 8], F32, bufs=1)
            scr = work.tile([128, 512], F32, tag="scr", bufs=2, name="scr")
            for s in range(8):
                ns = slice(s * 512, (s + 1) * 512)
                nc.sync.dma_start(out=x8[:, :, ns], in_=x8r[:, :, ns])
                for t in range(3):
                    nc.vector.bn_stats(out=stats[:, t, s, :], in_=x8[:, t, ns])
                for t in range(3, 4):
                    scr_t = work.tile([128, 512], F32, tag="scr", bufs=2, name="scr")
                    nc.scalar.activation(out=scr_t, in_=x8[:, t, ns], func=AF.Copy,
                                         accum_out=sums[:, t - 3, 0, s:s + 1])
                    scr_t2 = work.tile([128, 512], F32, tag="scr", bufs=2, name="scr")
                    nc.scalar.activation(out=scr_t2, in_=x8[:, t, ns], func=AF.Square,
                                         accum_out=sums[:, t - 3, 1, s:s + 1])
            mv = work.tile([128, 3, 2], F32, bufs=1)
            for t in range(3):
                nc.vector.bn_aggr(out=mv[:, t, :], in_=stats[:, t, :, :])
            # per-channel [mean, E[x^2]]
            stats2 = work.tile([128, 4, 2], F32R, bufs=1)
            msq = work.tile([128, 3, 1], F32, bufs=1)
            nc.vector.tensor_copy(out=stats2[:, 0:3, 0:1], in_=mv[:, :, 0:1])
            nc.vector.tensor_mul(msq, mv[:, :, 0:1], mv[:, :, 0:1])
            nc.vector.tensor_add(stats2[:, 0:3, 1:2], mv[:, :, 1:2], msq)
            ssum = work.tile([128, 1, 2], F32, bufs=1)
            nc.vector.tensor_reduce(out=ssum.rearrange("p a b -> p (a b)").unsqueeze(2),
                                    in_=sums.rearrange("p a b s -> p (a b) s"),
                                    axis=mybir.AxisListType.X, op=ALU.add)
            nc.vector.tensor_scalar_mul(out=stats2[:, 3:4, :],
                                        in0=ssum.rearrange("p a b -> p a b"),
                                        scalar1=1.0 / 4096.0)
            # group stats via mask matmul: [32, 2] = (mean_g, E[x^2]_g)
            gps = ps.tile([32, 2], F32, tag="apl")
            for t in range(4):
                nc.tensor.matmul(
                    gps, sb_gmask[:, t, :], stats2[:, t, :],
                    start=(t == 0), stop=(t == 3),
                )
            gs = work.tile([32, 2], F32, bufs=1)
            nc.vector.tensor_copy(out=gs, in_=gps)
            msqg = work.tile([32, 1], F32, bufs=1)
            varg = work.tile([32, 1], F32, bufs=1)
            nc.vector.tensor_mul(msqg, gs[:, 0:1], gs[:, 0:1])
            nc.vector.tensor_sub(varg, gs[:, 1:2], msqg)
            # rstd = exp(-0.5*ln(var+eps))
            lng = work.tile([32, 1], F32, bufs=1)
            nc.scalar.activation(out=lng, in_=varg, func=AF.Ln, bias=eps32, scale=1.0)
            rstdg = work.tile([32, 1], F32, bufs=1)
            nc.scalar.activation(out=rstdg, in_=lng, func=AF.Exp, scale=-0.5)
            gstats2 = work.tile([32, 2], F32R, bufs=1)
            nc.vector.tensor_copy(out=gstats2[:, 0:1], in_=gs[:, 0:1])
            nc.vector.tensor_copy(out=gstats2[:, 1:2], in_=rstdg)

            # ---- per-channel affine A, Bs  (hid = x*A + Bs) ----
            A_all = work.tile([128, 4], F32, bufs=1)
            Bcol = work.tile([128, 4, 2], F32R, bufs=1)
            for t in range(4):
                cst = ps.tile([128, 2], F32, tag="sc", bufs=3)
                nc.tensor.matmul(
                    cst, sb_bmask[:, t, :], gstats2, start=True, stop=True
                )
                nc.vector.tensor_mul(A_all[:, t:t + 1], cst[:, 1:2], sb_gamma[:, t:t + 1])
                tmp = work.tile([128, 1], F32, tag="tmp")
                nc.vector.tensor_mul(tmp, cst[:, 0:1], A_all[:, t:t + 1])
                nc.vector.tensor_sub(Bcol[:, t, :], sb_beta[:, t:t + 1].broadcast_to([128, 2]), tmp.broadcast_to([128, 2]))

            # ---- fold affine into QKV weights ----
            # bias' = W^T @ Bs + b (reads original f32r W), then fp8 W' = W*A
            cq_ps = ps.tile([128, 2], F32, tag="sc", bufs=3)
            ck_ps = ps.tile([128, 2], F32, tag="apl")
            cv_ps = ps.tile([1, 130], F32, tag="apl")
            for t in range(4):
                nc.tensor.matmul(cq_ps, sb_wq[:, t, :], Bcol[:, t, :],
                                 start=(t == 0), stop=(t == 3))
                nc.tensor.matmul(ck_ps, sb_wk[:, t, :], Bcol[:, t, :],
                                 start=(t == 0), stop=(t == 3))
                nc.tensor.matmul(cv_ps, Bcol[:, t, 0:1], sb_wv[:, t, :],
                                 start=(t == 0), stop=(t == 3))
            qc = consts.tile([128, 1], F32)
            nc.vector.tensor_add(qc, cq_ps[:, 0:1], sb_qb)
            kc = consts.tile([128, 1], F32)
            nc.vector.tensor_add(kc, ck_ps[:, 0:1], sb_kb)
            kc0 = consts.tile([128, 1], F32)
            nc.vector.tensor_mul(kc0, kc, mh0)
            kc1 = consts.tile([128, 1], F32)
            nc.vector.tensor_mul(kc1, kc, mh1)
            vrow = work.tile([1, 130], F32, bufs=1)
            nc.vector.tensor_add(vrow, cv_ps[:, 0:130], sb_vb)
            vbc = consts.tile([128, 130], F32)
            nc.gpsimd.partition_broadcast(vbc, vrow)
            # fp8 folded weights: [p, d(2), i(2), m]
            wq8 = consts.tile([128, 2, 2, 128], F8)
            wk8 = consts.tile([128, 2, 2, 128], F8)
            wv8 = consts.tile([128, 2, 2, 130], F8)
            for t in range(4):
                d, i = t // 2, t % 2
                nc.vector.tensor_scalar_mul(
                    out=wq8[:, d, i, :], in0=_f(sb_wq[:, t, :]), scalar1=A_all[:, t:t + 1])
                nc.vector.tensor_scalar_mul(
                    out=wk8[:, d, i, :], in0=_f(sb_wk[:, t, :]), scalar1=A_all[:, t:t + 1])
                nc.vector.tensor_scalar_mul(
                    out=wv8[:, d, i, :], in0=_f(sb_wv[:, t, :]), scalar1=A_all[:, t:t + 1])

            # ---- QKV (fp8 DoubleRow: contraction 512 = 2 passes x 256) ----
            q2 = big.tile([128, L], F32R)
            k2z = [big.tile([128, L], F32R, name="k2z0"),
                   big.tile([128, L], F32R, name="k2z1")]
            for n in range(8):
                ns = slice(n * 512, (n + 1) * 512)
                qp = ps.tile([128, 512], F32, tag="sc", bufs=3, name="qp")
                for d in range(2):
                    nc.tensor.matmul(qp, wq8[:, d], x8[:, 2 * d:2 * d + 2, ns],
                                     start=(d == 0), stop=(d == 1),
                                     perf_mode=PM.DoubleRow)
                if n % 2 == 0:
                    nc.scalar.activation(out=q2[:, ns], in_=qp, func=AF.Identity,
                                         bias=qc, scale=1.0)
                else:
                    nc.vector.tensor_scalar_add(out=q2[:, ns], in0=qp, scalar1=qc)
                kp = ps.tile([128, 512], F32, tag="sc", bufs=3, name="kp")
                for d in range(2):
                    nc.tensor.matmul(kp, wk8[:, d], x8[:, 2 * d:2 * d + 2, ns],
                                     start=(d == 0), stop=(d == 1),
                                     perf_mode=PM.DoubleRow)
                # (k + kc) masked per head (scores contract K=128)
                if n % 2 == 0:
                    nc.scalar.activation(out=k2z[0][:, ns], in_=kp, func=AF.Identity,
                                         bias=kc0, scale=mh0)
                    nc.vector.tensor_scalar(out=k2z[1][:, ns], in0=kp, scalar1=kc,
                                            scalar2=mh1, op0=ALU.add, op1=ALU.mult)
                else:
                    nc.vector.tensor_scalar(out=k2z[0][:, ns], in0=kp, scalar1=kc,
                                            scalar2=mh0, op0=ALU.add, op1=ALU.mult)
                    nc.scalar.activation(out=k2z[1][:, ns], in_=kp, func=AF.Identity,
                                         bias=kc1, scale=mh1)

            def emit_vp(j):
                js = slice(j * 128, (j + 1) * 128)
                vp = ps.tile([128, 130], F32, tag="sc", bufs=3, name="vp")
                for d in range(2):
                    nc.tensor.matmul(vp, x8[:, 2 * d:2 * d + 2, js], wv8[:, d],
                                     start=(d == 0), stop=(d == 1),
                                     perf_mode=PM.DoubleRow)
                m, i = j // 2, j % 2
                # heads' 65-col blocks (v + ones col) into padded stationary
                nc.vector.tensor_add(
                    vt8[:, m, i, :, 0:65],
                    vp.rearrange("p (h c) -> p h c", h=2),
                    vbc.rearrange("p (h c) -> p h c", h=2))

            for j in range(SJ):
                emit_vp(j)

            # ---- attention ----
            a_cat = big.tile([128, L], F32R, tag="xt")
            dbat = work.tile([8, 128], F32, tag="dbat", bufs=2, name="dbat")
            rrow = work.tile([1, TSUP], F32, tag="rrow", bufs=2, name="rrow")

            def emit_normalize(key, acp_t):
                hh, ts_idx = key
                tb = ts_idx * TSUP
                hsn = slice(CH * hh, CH * (hh + 1))
                rt = work.tile([8, 128], F32, tag="rt", bufs=2, name="rt")
                nc.vector.reciprocal(rt, dbat)
                nc.sync.dma_start(
                    out=rrow.rearrange("o (p f) -> o p f", p=8), in_=rt)
                rbc = work.tile([64, TSUP], F32, tag="rbc", bufs=2, name="rbc")
                for g in range(2):
                    gsl = slice(g * 512, (g + 1) * 512)
                    nc.gpsimd.partition_broadcast(rbc[:, gsl], rrow[:, gsl])
                    nc.gpsimd.tensor_mul(
                        a_cat[hsn, tb + g * 512:tb + (g + 1) * 512],
                        acp_t[0:64, gsl], rbc[:, gsl])

            def emit_proj_piece(ts_idx, piece):
                # piece 0..7 -> (n, m): n-outer so the first a_cat half suffices
                tb = ts_idx * TSUP
                n, m = piece // 4, piece % 4
                ms = slice(m * 128, (m + 1) * 128)
                ns = slice(tb + n * 512, tb + (n + 1) * 512)
                pp = ps.tile([128, 512], F32, tag="sc", bufs=3, name="pp")
                nc.tensor.matmul(pp, sb_pw[:, ms], a_cat[:, ns],
                                 start=True, stop=True)
                pt = work.tile([128, 512], F32, tag="pt", bufs=2, name="pt")
                if m % 2 == 0:
                    nc.scalar.activation(out=pt, in_=pp, func=AF.Copy)
                else:
                    nc.vector.tensor_copy(out=pt, in_=pp)
                nc.sync.dma_start(out=part[ms, ns], in_=pt)

            pending_norm = None
            for tsup in range(NT):
                t0 = tsup * TSUP
                for h in range(HEADS_PER_CORE):
                    apl = ps.tile([128, TSUP], F32, tag="apl", name="apl")
                    E8s = {}
                    E8 = None
                    for j in range(SJ + 4):
                        if j == 4 and pending_norm is not None:
                            emit_normalize(*pending_norm)
                            pending_norm = None
                        if h == 1 and tsup > 0 and j in (17, 19, 21, 23, 25, 27, 29, 31):
                            emit_proj_piece(tsup - 1, (j - 17) // 2)
                        if j < SJ:
                            js = slice(j * 128, (j + 1) * 128)
                            if j % 2 == 0:
                                E8s[j // 2] = work.tile([128, 2, TSUP], F8, bufs=4, name="E8")
                            E8 = E8s[j // 2]
                            sc = ps.tile([128, TSUP], F32, tag="sc", bufs=3, name="sc")
                            nc.tensor.matmul(sc[:, 0:512], k2z[h][:, js],
                                             q2[:, t0:t0 + 512], start=True, stop=True)
                            nc.tensor.matmul(sc[:, 512:1024], k2z[h][:, js],
                                             q2[:, t0 + 512:t0 + 1024],
                                             start=True, stop=True)
                            pidx = j // 2
                            on_dve = (pidx % 2 == 0) == (j % 2 == 1)
                            if on_dve:
                                nc.vector.tensor_scalar(
                                    out=E8[:, j % 2, :].bitcast(U8), in0=sc,
                                    scalar1=SCH_A, scalar2=SCH_B,
                                    op0=ALU.mult, op1=ALU.add)
                            else:
                                nc.scalar.activation(
                                    out=E8[:, j % 2, :], in_=sc, func=AF.Exp,
                                    scale=0.125, bias=ebias_t)
                        # av lagged two pairs
                        if j % 2 == 1 and j >= 5:
                            pav = (j - 5) // 2
                            Ep = E8s.pop(pav)
                            vst = vt8[:, pav, :, h, :]
                            nc.tensor.matmul(apl[:, 0:512], vst, Ep[:, :, 0:512],
                                             start=(pav == 0), stop=(pav == 15),
                                             perf_mode=PM.DoubleRow)
                            nc.tensor.matmul(apl[:, 512:1024], vst, Ep[:, :, 512:1024],
                                             start=(pav == 0), stop=(pav == 15),
                                             perf_mode=PM.DoubleRow)
                    # move a_plus off PSUM (rows 0:64 = channels, 64 = denom)
                    acp = work.tile([65, TSUP], F32, tag="acp", bufs=3, name="acp")
                    nc.scalar.activation(out=acp, in_=apl[0:65, :], func=AF.Copy)
                    # denominator row -> [8, 128] for partition-parallel recip
                    nc.sync.dma_start(
                        out=dbat,
                        in_=acp[64:65, :].rearrange("o (p f) -> o p f", p=8))
                    pending_norm = ((h, tsup), acp)
            emit_normalize(*pending_norm)
            for piece in range(8):
                emit_proj_piece(NT - 1, piece)

    nc.compile()
    return nc


def get_program():
    global _PROGRAM
    if _PROGRAM is None:
        _PROGRAM = build_program()
    return _PROGRAM


def make_in_maps(x, norm_w, norm_b, qkv_w, qkv_b, proj_w):
    """Build the 8 per-core input maps from full inputs."""
    import ml_dtypes
    f = np.float32
    x8 = np.ascontiguousarray(x.reshape(B, C, L)).astype(ml_dtypes.float8_e4m3fn)

    gmask = np.zeros((128, 4, G), dtype=f)
    bmask = np.zeros((G, 4, 128), dtype=f)
    for t in range(4):
        for p in range(128):
            g = (t * 128 + p) // 16
            gmask[p, t, g] = 1.0 / 16.0
            bmask[g, t, p] = 1.0
    gamma4 = np.ascontiguousarray(norm_w.reshape(4, 128), dtype=f)
    beta4 = np.ascontiguousarray(norm_b.reshape(4, 128), dtype=f)

    in_maps = []
    for cid in range(N_CORES):
        b = cid // 4
        h0 = 2 * (cid % 4)
        h1 = h0 + 1
        qrows = list(range(192 * h0, 192 * h0 + 64)) + list(range(192 * h1, 192 * h1 + 64))
        krows = [r + 64 for r in qrows]
        v0 = list(range(192 * h0 + 128, 192 * h0 + 192))
        v1 = list(range(192 * h1 + 128, 192 * h1 + 192))
        wqT = np.ascontiguousarray(qkv_w[qrows, :].T, dtype=f)
        wkT = np.ascontiguousarray(qkv_w[krows, :].T, dtype=f)
        wvT = np.zeros((C, 130), dtype=f)
        wvT[:, 0:64] = qkv_w[v0, :].T
        wvT[:, 65:129] = qkv_w[v1, :].T
        qbv = np.ascontiguousarray(qkv_b[qrows], dtype=f)
        kbv = np.ascontiguousarray(qkv_b[krows], dtype=f)
        vbv = np.zeros((130,), dtype=f)
        vbv[0:64] = qkv_b[v0]
        vbv[65:129] = qkv_b[v1]
        vbv[64] = 1.0    # softmax-denominator ones columns (weight cols there are 0)
        vbv[129] = 1.0
        ch_cols = list(range(64 * h0, 64 * h0 + 64)) + list(range(64 * h1, 64 * h1 + 64))
        pwT = np.ascontiguousarray(proj_w[:, ch_cols].T, dtype=f)
        in_maps.append({
            "x8b": x8[b], "gmask": gmask, "bmask": bmask,
            "gamma4": gamma4, "beta4": beta4,
            "wqT": wqT, "wkT": wkT, "wvT": wvT,
            "qb": qbv, "kb": kbv, "vb": vbv, "pwT": pwT,
        })
    return in_maps


def kernel(x, norm_w, norm_b, qkv_w, qkv_b, proj_w, proj_b, _trace=False):
    x = np.asarray(x, dtype=np.float32)
    in_maps = make_in_maps(x, np.asarray(norm_w), np.asarray(norm_b),
                           np.asarray(qkv_w), np.asarray(qkv_b), np.asarray(proj_w))
    nc = get_program()
    res = run_bass_kernel_spmd(nc, in_maps, list(range(N_CORES)), trace=_trace)
    hout = np.zeros((B, C, L), dtype=np.float32)
    for cid in range(N_CORES):
        hout[cid // 4] += res.results[cid]["part"]
    hout += np.asarray(proj_b, dtype=np.float32)[None, :, None]
    out = x + hout.reshape(x.shape)
    if _trace:
        return out.astype(np.float32), res
    return out.astype(np.float32)


# revision 14
# speedup vs baseline: 1.0442x; 1.0145x over previous
"""AttentionBlock (GroupNorm + QKV + 8-head spatial attention + proj + residual)
on 8 Trainium2 NeuronCores.

Sharding: 16 head-batches (B=2 x NH=8) split 2-per-core; cores 0-3 take batch
0, cores 4-7 batch 1.  Per core:
  - x arrives as fp8e4m3 [512, 4096]; GroupNorm statistics computed on-chip
    (bn_stats per channel, group-combine via tiny mask matmuls on the PE),
  - GroupNorm affine folded into the QKV weights (W' = W*A per channel,
    bias' = W@B + qkv_b); QKV matmuls run in fp8 DoubleRow mode (contracting
    2x128 channels per pass),
  - scores in f32r [s,t] layout (K=128 with the other head's k rows zeroed),
  - exp is split between the ACT engine (Exp -> fp8 out) and the DVE
    (Schraudolph: bits = rint(A*sc+B) saturating to uint8, bit-identical
    cost, ~3% sawtooth err) writing E as fp8e4m3 pairs [128, 2, 1024],
  - a_plus = vT @ E in fp8 DoubleRow mode (contracts 2 s-chunks of 128 per
    pass; v transposed out of QKV with an extra ones-column so softmax
    denominators are free; av lags exp by one pair so the PE never stalls),
  - per-t normalization via partition-scattered reciprocal (DMA reshapes the
    denominator row to [8,128] so DVE reciprocal uses partition parallelism),
  - partial projection proj_w[:, head_cols] @ a emitted per t-stripe, lagged.
Host sums the 4 partials per batch, adds proj_b + proj_w@vbias' term... (vb
is kept on-device in vT, so host only adds proj_b) and the residual.
"""

import math
import numpy as np

import concourse.bacc as bacc
import concourse.tile as tile
from concourse import mybir
from concourse.bass_utils import run_bass_kernel_spmd

B, C = 2, 512
L = 64 * 64           # 4096
NH = 8                # heads total
CH = 64               # channels per head
G = 32                # groups
EPS = 1e-5
N_CORES = 8
HEADS_PER_CORE = 2

F32 = mybir.dt.float32
F32R = mybir.dt.float32r
F8 = mybir.dt.float8e4
U8 = mybir.dt.uint8
AF = mybir.ActivationFunctionType
ALU = mybir.AluOpType
PM = mybir.MatmulPerfMode

TSUP = 1024           # t-stripe width
NT = L // TSUP        # 4 stripes
SJ = 32               # number of 128-wide s-chunks

# exp-domain shift (softmax-invariant; keeps fp8 E in range)
EBIAS = 2.5
# DVE schraudolph constants: bits = rint(sc*SCH_A + SCH_B), sc = raw q.k
SCH_A = 8.0 * (1.0 / math.log(2.0)) * 0.125
SCH_B = 56.0 - 8.0 * EBIAS * (1.0 / math.log(2.0)) - 2.8

# which j-chunks the DVE computes (rest on ACT); ~37.5%
DVE_J = frozenset((1, 4, 7, 10, 12, 15))


def _f(ap):
    return ap.bitcast(F32)


_PROGRAM = None


def build_program():
    nc = bacc.Bacc()
    x8b = nc.declare_dram_parameter("x8b", [C, L], F8, isOutput=False).ap()
    gmask = nc.declare_dram_parameter("gmask", [128, 4, G], F32R, isOutput=False).ap()
    bmask = nc.declare_dram_parameter("bmask", [G, 4, 128], F32R, isOutput=False).ap()
    gamma4 = nc.declare_dram_parameter("gamma4", [4, 128], F32, isOutput=False).ap()
    beta4 = nc.declare_dram_parameter("beta4", [4, 128], F32, isOutput=False).ap()
    wqT = nc.declare_dram_parameter("wqT", [C, 128], F32R, isOutput=False).ap()
    wkT = nc.declare_dram_parameter("wkT", [C, 128], F32R, isOutput=False).ap()
    wvT = nc.declare_dram_parameter("wvT", [C, 130], F32R, isOutput=False).ap()
    qb = nc.declare_dram_parameter("qb", [128], F32, isOutput=False).ap()
    kb = nc.declare_dram_parameter("kb", [128], F32, isOutput=False).ap()
    vb = nc.declare_dram_parameter("vb", [130], F32, isOutput=False).ap()
    pwT = nc.declare_dram_parameter("pwT", [128, C], F32R, isOutput=False).ap()
    part = nc.declare_dram_parameter("part", [C, L], F32, isOutput=True).ap()

    with tile.TileContext(nc) as tc:
        with (
            tc.tile_pool(name="consts", bufs=1) as consts,
            tc.tile_pool(name="big", bufs=1) as big,
            tc.tile_pool(name="work", bufs=2) as work,
            tc.tile_pool(name="ps", bufs=1, space="PSUM") as ps,
        ):
            # ---- constants into SBUF ----
            sb_gmask = consts.tile([128, 4, G], F32R)
            nc.gpsimd.dma_start(out=sb_gmask, in_=gmask)
            sb_bmask = consts.tile([G, 4, 128], F32R)
            nc.gpsimd.dma_start(out=sb_bmask, in_=bmask)
            sb_gamma = consts.tile([128, 4], F32)
            nc.gpsimd.dma_start(out=sb_gamma, in_=gamma4.rearrange("t p -> p t"))
            sb_beta = consts.tile([128, 4], F32)
            nc.gpsimd.dma_start(out=sb_beta, in_=beta4.rearrange("t p -> p t"))
            sb_wq = consts.tile([128, 4, 128], F32R)
            nc.gpsimd.dma_start(out=sb_wq, in_=wqT.rearrange("(kk p) m -> p kk m", p=128))
            sb_wk = consts.tile([128, 4, 128], F32R)
            nc.gpsimd.dma_start(out=sb_wk, in_=wkT.rearrange("(kk p) m -> p kk m", p=128))
            sb_wv = consts.tile([128, 4, 130], F32R)
            nc.gpsimd.dma_start(out=sb_wv, in_=wvT.rearrange("(kk p) m -> p kk m", p=128))
            sb_pw = consts.tile([128, C], F32R)
            nc.gpsimd.dma_start(out=sb_pw, in_=pwT)
            sb_qb = consts.tile([128, 1], F32)
            nc.gpsimd.dma_start(out=sb_qb, in_=qb.unsqueeze(1))
            sb_kb = consts.tile([128, 1], F32)
            nc.gpsimd.dma_start(out=sb_kb, in_=kb.unsqueeze(1))
            sb_vb = consts.tile([1, 130], F32)
            nc.gpsimd.dma_start(out=sb_vb, in_=vb.unsqueeze(0))
            eps32 = consts.tile([32, 1], F32)
            nc.vector.memset(eps32, EPS)
            ebias_t = consts.tile([128, 1], F32)
            nc.vector.memset(ebias_t, -EBIAS)
            mh0 = consts.tile([128, 1], F32)
            nc.vector.memset(mh0[0:64, :], 1.0)
            nc.vector.memset(mh0[64:128, :], 0.0)
            mh1 = consts.tile([128, 1], F32)
            nc.vector.memset(mh1[0:64, :], 0.0)
            nc.vector.memset(mh1[64:128, :], 1.0)

            # vt8: av stationary, [p, m(16), i(2), h(2), 128]; cols 0:64 v,
            # col 64 ones (denominator), 65:127 zero-pad
            vt8 = big.tile([128, 16, 2, 2, 128], F8)
            nc.gpsimd.memset(vt8[:, :, :, :, 65:128], 0.0)

            # ---- load x8 ----
            x8 = big.tile([128, 4, L], F8)
            x8r = x8b.rearrange("(t p) l -> p t l", p=128)
            stats = work.tile([128, 3, 8, 6], F32, bufs=1)
            sums = work.tile([128, 1, 2, 8], F32, bufs=1)
            scr = work.tile([128, 512], F32, tag="scr", bufs=2, name="scr")
            for s in range(8):
                ns = slice(s * 512, (s + 1) * 512)
                nc.sync.dma_start(out=x8[:, :, ns], in_=x8r[:, :, ns])
                for t in range(3):
                    nc.vector.bn_stats(out=stats[:, t, s, :], in_=x8[:, t, ns])
                for t in range(3, 4):
                    scr_t = work.tile([128, 512], F32, tag="scr", bufs=2, name="scr")
                    nc.scalar.activation(out=scr_t, in_=x8[:, t, ns], func=AF.Copy,
                                         accum_out=sums[:, t - 3, 0, s:s + 1])
                    scr_t2 = work.tile([128, 512], F32, tag="scr", bufs=2, name="scr")
                    nc.scalar.activation(out=scr_t2, in_=x8[:, t, ns], func=AF.Square,
                                         accum_out=sums[:, t - 3, 1, s:s + 1])
            mv = work.tile([128, 3, 2], F32, bufs=1)
            for t in range(3):
                nc.vector.bn_aggr(out=mv[:, t, :], in_=stats[:, t, :, :])
            # per-channel [mean, E[x^2]]
            stats2 = work.tile([128, 4, 2], F32R, bufs=1)
            msq = work.tile([128, 3, 1], F32, bufs=1)
            nc.vector.tensor_copy(out=stats2[:, 0:3, 0:1], in_=mv[:, :, 0:1])
            nc.vector.tensor_mul(msq, mv[:, :, 0:1], mv[:, :, 0:1])
            nc.vector.tensor_add(stats2[:, 0:3, 1:2], mv[:, :, 1:2], msq)
            ssum = work.tile([128, 1, 2], F32, bufs=1)
            nc.vector.tensor_reduce(out=ssum.rearrange("p a b -> p (a b)").unsqueeze(2),
                                    in_=sums.rearrange("p a b s -> p (a b) s"),
                                    axis=mybir.AxisListType.X, op=ALU.add)
            nc.vector.tensor_scalar_mul(out=stats2[:, 3:4, :],
                                        in0=ssum.rearrange("p a b -> p a b"),
                                        scalar1=1.0 / 4096.0)
            # group stats via mask matmul: [32, 2] = (mean_g, E[x^2]_g)
            gps = ps.tile([32, 2], F32, tag="apl")
            for t in range(4):
                nc.tensor.matmul(
                    gps, sb_gmask[:, t, :], stats2[:, t, :],
                    start=(t == 0), stop=(t == 3),
                )
            gs = work.tile([32, 2], F32, bufs=1)
            nc.vector.tensor_copy(out=gs, in_=gps)
            msqg = work.tile([32, 1], F32, bufs=1)
            varg = work.tile([32, 1], F32, bufs=1)
            nc.vector.tensor_mul(msqg, gs[:, 0:1], gs[:, 0:1])
            nc.vector.tensor_sub(varg, gs[:, 1:2], msqg)
            # rstd = exp(-0.5*ln(var+eps))
            lng = work.tile([32, 1], F32, bufs=1)
            nc.scalar.activation(out=lng, in_=varg, func=AF.Ln, bias=eps32, scale=1.0)
            rstdg = work.tile([32, 1], F32, bufs=1)
            nc.scalar.activation(out=rstdg, in_=lng, func=AF.Exp, scale=-0.5)
            gstats2 = work.tile([32, 2], F32R, bufs=1)
            nc.vector.tensor_copy(out=gstats2[:, 0:1], in_=gs[:, 0:1])
            nc.vector.tensor_copy(out=gstats2[:, 1:2], in_=rstdg)

            # ---- per-channel affine A, Bs  (hid = x*A + Bs) ----
            A_all = work.tile([128, 4], F32, bufs=1)
            Bcol = work.tile([128, 4, 2], F32R, bufs=1)
            for t in range(4):
                cst = ps.tile([128, 2], F32, tag="sc", bufs=3)
                nc.tensor.matmul(
                    cst, sb_bmask[:, t, :], gstats2, start=True, stop=True
                )
                nc.vector.tensor_mul(A_all[:, t:t + 1], cst[:, 1:2], sb_gamma[:, t:t + 1])
                tmp = work.tile([128, 1], F32, tag="tmp")
                nc.vector.tensor_mul(tmp, cst[:, 0:1], A_all[:, t:t + 1])
                nc.vector.tensor_sub(Bcol[:, t, :], sb_beta[:, t:t + 1].broadcast_to([128, 2]), tmp.broadcast_to([128, 2]))

            # ---- fold affine into QKV weights ----
            # bias' = W^T @ Bs + b (reads original f32r W), then fp8 W' = W*A
            cq_ps = ps.tile([128, 2], F32, tag="sc", bufs=3)
            ck_ps = ps.tile([128, 2], F32, tag="apl")
            cv_ps = ps.tile([1, 130], F32, tag="apl")
            for t in range(4):
                nc.tensor.matmul(cq_ps, sb_wq[:, t, :], Bcol[:, t, :],
                                 start=(t == 0), stop=(t == 3))
                nc.tensor.matmul(ck_ps, sb_wk[:, t, :], Bcol[:, t, :],
                                 start=(t == 0), stop=(t == 3))
                nc.tensor.matmul(cv_ps, Bcol[:, t, 0:1], sb_wv[:, t, :],
                                 start=(t == 0), stop=(t == 3))
            qc = consts.tile([128, 1], F32)
            nc.vector.tensor_add(qc, cq_ps[:, 0:1], sb_qb)
            kc = consts.tile([128, 1], F32)
            nc.vector.tensor_add(kc, ck_ps[:, 0:1], sb_kb)
            kc0 = consts.tile([128, 1], F32)
            nc.vector.tensor_mul(kc0, kc, mh0)
            kc1 = consts.tile([128, 1], F32)
            nc.vector.tensor_mul(kc1, kc, mh1)
            vrow = work.tile([1, 130], F32, bufs=1)
            nc.vector.tensor_add(vrow, cv_ps[:, 0:130], sb_vb)
            vbc = consts.tile([128, 130], F32)
            nc.gpsimd.partition_broadcast(vbc, vrow)
            # fp8 folded weights: [p, d(2), i(2), m]
            wq8 = consts.tile([128, 2, 2, 128], F8)
            wk8 = consts.tile([128, 2, 2, 128], F8)
            wv8 = consts.tile([128, 2, 2, 130], F8)
            for t in range(4):
                d, i = t // 2, t % 2
                nc.vector.tensor_scalar_mul(
                    out=wq8[:, d, i, :], in0=_f(sb_wq[:, t, :]), scalar1=A_all[:, t:t + 1])
                nc.vector.tensor_scalar_mul(
                    out=wk8[:, d, i, :], in0=_f(sb_wk[:, t, :]), scalar1=A_all[:, t:t + 1])
                nc.vector.tensor_scalar_mul(
                    out=wv8[:, d, i, :], in0=_f(sb_wv[:, t, :]), scalar1=A_all[:, t:t + 1])

            # ---- QKV (fp8 DoubleRow: contraction 512 = 2 passes x 256) ----
            def emit_vp(j):
                js = slice(j * 128, (j + 1) * 128)
                vp = ps.tile([128, 130], F32, tag="sc", bufs=3, name="vp")
                for d in range(2):
                    nc.tensor.matmul(vp, x8[:, 2 * d:2 * d + 2, js], wv8[:, d],
                                     start=(d == 0), stop=(d == 1),
                                     perf_mode=PM.DoubleRow)
                m, i = j // 2, j % 2
                nc.vector.tensor_add(
                    vt8[:, m, i, :, 0:65],
                    vp.rearrange("p (h c) -> p h c", h=2),
                    vbc.rearrange("p (h c) -> p h c", h=2))

            q2 = big.tile([128, L], F32R)
            k2z = [big.tile([128, L], F32R, name="k2z0"),
                   big.tile([128, L], F32R, name="k2z1")]
            for n in range(8):
                ns = slice(n * 512, (n + 1) * 512)
                qp = ps.tile([128, 512], F32, tag="sc", bufs=3, name="qp")
                for d in range(2):
                    nc.tensor.matmul(qp, wq8[:, d], x8[:, 2 * d:2 * d + 2, ns],
                                     start=(d == 0), stop=(d == 1),
                                     perf_mode=PM.DoubleRow)
                if n % 2 == 0:
                    nc.scalar.activation(out=q2[:, ns], in_=qp, func=AF.Identity,
                                         bias=qc, scale=1.0)
                else:
                    nc.vector.tensor_scalar_add(out=q2[:, ns], in0=qp, scalar1=qc)
                kp = ps.tile([128, 512], F32, tag="sc", bufs=3, name="kp")
                for d in range(2):
                    nc.tensor.matmul(kp, wk8[:, d], x8[:, 2 * d:2 * d + 2, ns],
                                     start=(d == 0), stop=(d == 1),
                                     perf_mode=PM.DoubleRow)
                # (k + kc) masked per head (scores contract K=128)
                if n % 2 == 0:
                    nc.scalar.activation(out=k2z[0][:, ns], in_=kp, func=AF.Identity,
                                         bias=kc0, scale=mh0)
                    nc.vector.tensor_scalar(out=k2z[1][:, ns], in0=kp, scalar1=kc,
                                            scalar2=mh1, op0=ALU.add, op1=ALU.mult)
                else:
                    nc.vector.tensor_scalar(out=k2z[0][:, ns], in0=kp, scalar1=kc,
                                            scalar2=mh0, op0=ALU.add, op1=ALU.mult)
                    nc.scalar.activation(out=k2z[1][:, ns], in_=kp, func=AF.Identity,
                                         bias=kc1, scale=mh1)
                for jj in range(4 * n, 4 * n + 4):
                    emit_vp(jj)


            # ---- attention ----
            a_cat = big.tile([128, L], F32R, tag="xt")
            dbat = work.tile([8, 128], F32, tag="dbat", bufs=2, name="dbat")
            rrow = work.tile([1, TSUP], F32, tag="rrow", bufs=2, name="rrow")

            def emit_normalize(key, acp_t):
                hh, ts_idx = key
                tb = ts_idx * TSUP
                hsn = slice(CH * hh, CH * (hh + 1))
                rt = work.tile([8, 128], F32, tag="rt", bufs=2, name="rt")
                nc.vector.reciprocal(rt, dbat)
                nc.sync.dma_start(
                    out=rrow.rearrange("o (p f) -> o p f", p=8), in_=rt)
                rbc = work.tile([64, TSUP], F32, tag="rbc", bufs=2, name="rbc")
                for g in range(2):
                    gsl = slice(g * 512, (g + 1) * 512)
                    nc.gpsimd.partition_broadcast(rbc[:, gsl], rrow[:, gsl])
                    nc.gpsimd.tensor_mul(
                        a_cat[hsn, tb + g * 512:tb + (g + 1) * 512],
                        acp_t[0:64, gsl], rbc[:, gsl])

            def emit_proj_piece(ts_idx, piece):
                # piece 0..7 -> (n, m): n-outer so the first a_cat half suffices
                tb = ts_idx * TSUP
                n, m = piece // 4, piece % 4
                ms = slice(m * 128, (m + 1) * 128)
                ns = slice(tb + n * 512, tb + (n + 1) * 512)
                pp = ps.tile([128, 512], F32, tag="sc", bufs=3, name="pp")
                nc.tensor.matmul(pp, sb_pw[:, ms], a_cat[:, ns],
                                 start=True, stop=True)
                pt = work.tile([128, 512], F32, tag="pt", bufs=3, name="pt")
                if m % 2 == 0:
                    nc.scalar.activation(out=pt, in_=pp, func=AF.Copy)
                else:
                    nc.vector.tensor_copy(out=pt, in_=pp)
                nc.sync.dma_start(out=part[ms, ns], in_=pt)

            pending_norm = None
            for tsup in range(NT):
                t0 = tsup * TSUP
                for h in range(HEADS_PER_CORE):
                    apl = ps.tile([128, TSUP], F32, tag="apl", name="apl")
                    E8s = {}
                    E8 = None
                    for j in range(SJ + 4):
                        if j == 4 and pending_norm is not None:
                            emit_normalize(*pending_norm)
                            pending_norm = None
                        if h == 1 and tsup > 0 and j in (17, 19, 21, 23, 25, 27, 29, 31):
                            emit_proj_piece(tsup - 1, (j - 17) // 2)
                        if j < SJ:
                            js = slice(j * 128, (j + 1) * 128)
                            if j % 2 == 0:
                                E8s[j // 2] = work.tile([128, 2, TSUP], F8, bufs=4, name="E8")
                            E8 = E8s[j // 2]
                            sc = ps.tile([128, TSUP], F32, tag="sc", bufs=3, name="sc")
                            nc.tensor.matmul(sc[:, 0:512], k2z[h][:, js],
                                             q2[:, t0:t0 + 512], start=True, stop=True)
                            nc.tensor.matmul(sc[:, 512:1024], k2z[h][:, js],
                                             q2[:, t0 + 512:t0 + 1024],
                                             start=True, stop=True)
                            pidx = j // 2
                            on_dve = (pidx % 2 == 0) == (j % 2 == 1)
                            if on_dve:
                                nc.vector.tensor_scalar(
                                    out=E8[:, j % 2, :].bitcast(U8), in0=sc,
                                    scalar1=SCH_A, scalar2=SCH_B,
                                    op0=ALU.mult, op1=ALU.add)
                            else:
                                nc.scalar.activation(
                                    out=E8[:, j % 2, :], in_=sc, func=AF.Exp,
                                    scale=0.125, bias=ebias_t)
                        # av lagged two pairs
                        if j % 2 == 1 and j >= 5:
                            pav = (j - 5) // 2
                            Ep = E8s.pop(pav)
                            vst = vt8[:, pav, :, h, :]
                            nc.tensor.matmul(apl[:, 0:512], vst, Ep[:, :, 0:512],
                                             start=(pav == 0), stop=(pav == 15),
                                             perf_mode=PM.DoubleRow)
                            nc.tensor.matmul(apl[:, 512:1024], vst, Ep[:, :, 512:1024],
                                             start=(pav == 0), stop=(pav == 15),
                                             perf_mode=PM.DoubleRow)
                    # move a_plus off PSUM (rows 0:64 = channels, 64 = denom)
                    acp = work.tile([65, TSUP], F32, tag="acp", bufs=3, name="acp")
                    nc.scalar.activation(out=acp, in_=apl[0:65, :], func=AF.Copy)
                    # denominator row -> [8, 128] for partition-parallel recip
                    nc.sync.dma_start(
                        out=dbat,
                        in_=acp[64:65, :].rearrange("o (p f) -> o p f", p=8))
                    pending_norm = ((h, tsup), acp)
            emit_normalize(*pending_norm)
            for piece in range(8):
                emit_proj_piece(NT - 1, piece)

    nc.compile()
    return nc


def get_program():
    global _PROGRAM
    if _PROGRAM is None:
        _PROGRAM = build_program()
    return _PROGRAM


def make_in_maps(x, norm_w, norm_b, qkv_w, qkv_b, proj_w):
    """Build the 8 per-core input maps from full inputs."""
    import ml_dtypes
    f = np.float32
    x8 = np.ascontiguousarray(x.reshape(B, C, L)).astype(ml_dtypes.float8_e4m3fn)

    gmask = np.zeros((128, 4, G), dtype=f)
    bmask = np.zeros((G, 4, 128), dtype=f)
    for t in range(4):
        for p in range(128):
            g = (t * 128 + p) // 16
            gmask[p, t, g] = 1.0 / 16.0
            bmask[g, t, p] = 1.0
    gamma4 = np.ascontiguousarray(norm_w.reshape(4, 128), dtype=f)
    beta4 = np.ascontiguousarray(norm_b.reshape(4, 128), dtype=f)

    in_maps = []
    for cid in range(N_CORES):
        b = cid // 4
        h0 = 2 * (cid % 4)
        h1 = h0 + 1
        qrows = list(range(192 * h0, 192 * h0 + 64)) + list(range(192 * h1, 192 * h1 + 64))
        krows = [r + 64 for r in qrows]
        v0 = list(range(192 * h0 + 128, 192 * h0 + 192))
        v1 = list(range(192 * h1 + 128, 192 * h1 + 192))
        wqT = np.ascontiguousarray(qkv_w[qrows, :].T, dtype=f)
        wkT = np.ascontiguousarray(qkv_w[krows, :].T, dtype=f)
        wvT = np.zeros((C, 130), dtype=f)
        wvT[:, 0:64] = qkv_w[v0, :].T
        wvT[:, 65:129] = qkv_w[v1, :].T
        qbv = np.ascontiguousarray(qkv_b[qrows], dtype=f)
        kbv = np.ascontiguousarray(qkv_b[krows], dtype=f)
        vbv = np.zeros((130,), dtype=f)
        vbv[0:64] = qkv_b[v0]
        vbv[65:129] = qkv_b[v1]
        vbv[64] = 1.0    # softmax-denominator ones columns (weight cols there are 0)
        vbv[129] = 1.0
        ch_cols = list(range(64 * h0, 64 * h0 + 64)) + list(range(64 * h1, 64 * h1 + 64))
        pwT = np.ascontiguousarray(proj_w[:, ch_cols].T, dtype=f)
        in_maps.append({
            "x8b": x8[b], "gmask": gmask, "bmask": bmask,
            "gamma4": gamma4, "beta4": beta4,
            "wqT": wqT, "wkT": wkT, "wvT": wvT,
            "qb": qbv, "kb": kbv, "vb": vbv, "pwT": pwT,
        })
    return in_maps


def kernel(x, norm_w, norm_b, qkv_w, qkv_b, proj_w, proj_b, _trace=False):
    x = np.asarray(x, dtype=np.float32)
    in_maps = make_in_maps(x, np.asarray(norm_w), np.asarray(norm_b),
                           np.asarray(qkv_w), np.asarray(qkv_b), np.asarray(proj_w))
    nc = get_program()
    res = run_bass_kernel_spmd(nc, in_maps, list(range(N_CORES)), trace=_trace)
    hout = np.zeros((B, C, L), dtype=np.float32)
    for cid in range(N_CORES):
        hout[cid // 4] += res.results[cid]["part"]
    hout += np.asarray(proj_b, dtype=np.float32)[None, :, None]
    out = x + hout.reshape(x.shape)
    if _trace:
        return out.astype(np.float32), res
    return out.astype(np.float32)
